# revision 1
# baseline (speedup 1.0000x reference)
"""Trainium2 Bass kernel for BinOverlapPredictionFromMaxProj (segment max + masked mean).

Full computation:
  ptm: (32, 8, 30, 1, 72, 72) f32, mem_mask: (32, 8, 30) bool
  n = 32*8 = 256 rows; per row: max over 5184-feature axis per mem (30), then
  masked mean over mems -> out (256,) f32.

Sharding: data-parallel over the 256 fused rows across 8 cores (32 rows each).
Per core: 960 segments x 5184 features (~19.9 MB) -> memory-bound; the DMA
stream (19.9 MB at ~425 GB/s ~= 47 us) is the roofline.

Device plan per core (pair-aligned layout):
  The host pre-arranges each core's 1920 half-segments (960 segs x 2 halves of
  2592 floats) into a (128, 15, 2592) buffer so that
    - cols 0..13 of partition p hold 7 WHOLE segments (halves adjacent), and
    - col 14 holds one stray HALF; row r owns partitions 4r..4r+3, its two
      stray segments' halves sitting at col 14 of those four partitions.
  This makes the segment pair-max a within-partition stride-2 tensor_max and
  the cross-partition masked row-sum a single PE matmul (128->32 block-sum
  weights), eliminating the SBUF repartition DMA of the earlier design.

  Loads: col 14 first (w=1), then 6 col-pair loads (w=2) and the last
  segment as two w=1 loads (its half-maxes arrive earlier; the pipeline end
  is delivery-gated), all on the gpsimd SWDGE queue (HWDGE/sync measured ~2x
  slower for big loads; f32 keeps write packets >= 10 KB - smaller packets
  risk SDMA engine 15 falling ~10us behind, delaying every load's
  semaphore). All constants ride in TWO small DMAs: more small transfers
  mean hundreds of tiny descriptors that round-robin against the stream and
  take >15us to complete. Vector reduces each load as it lands (w=2 X-axis
  reduces, 2765 ns/col; the XY-axis variant measured 20% slower).

  Stray path (off critical path): statS (128,1) -> PE transpose -> (1,128)
  PSUM -> stride-2 max + mask + pair-sum on one partition -> straysum (1,32).
  Main tail: stride-2 pairmax -> (128,7), mask-mul, row-partial reduce,
  PE matmul partial.T @ W1 -> psum (1,32), add straysum, mul 1/count,
  single-descriptor 128B out DMA (a 1-D dram tensor paired with a
  1-partition SBUF AP mis-lowers, hence out is declared [1, 32]).

  Two early dummy DVE reduces ("burn") run in the pre-stream idle window:
  roughly half of the runs otherwise execute every DVE op ~20% slow (6.6us
  vs 5.5us per w=2 reduce); with the warm-up that state has not recurred.

The walrus codegen allows only ONE attached sync wait per instruction, so
TileContext's kernel-tail Drain is rewritten (waits beyond the first become
standalone wait_ge), PE "warmup" matmuls touch the constant tile so later
matmuls carry a single data wait, and a DVE touch-copy covers the const DMA
for the late mask-multiply. See _patch_tile_drain().
"""

import sys

import numpy as np

if "/opt/trn_rl_repo" not in sys.path:
    sys.path.insert(0, "/opt/trn_rl_repo")

NCORES = 8
NF, NS, NMEM, FEAT = 32, 8, 30, 5184
N = NF * NS  # 256
ROWS = N // NCORES  # 32 rows per core
SEGS = ROWS * NMEM  # 960 segments per core
PPART = 128  # partitions
HALF = FEAT // 2  # 2592 floats per half-segment
HPP = SEGS * 2 // PPART  # 15 half-segments per partition
NWHOLE = 7  # whole segments per partition (cols 0..13)

_NC_CACHE = {}


def _patch_tile_drain():
    """Split the kernel-tail Drain's semaphore waits into standalone wait_ge
    instructions (one wait per instruction), to fit the walrus per-instruction
    sync-wait limit."""
    import concourse.tile as tile
    from concourse.vector_clock import ScopedClock

    if getattr(tile.TileContext._drain_and_barrier, "_single_wait_patch", False):
        return

    def _drain_and_barrier(self, tick_clock, wait_clock):
        drain_inst = self.nc.sync.drain()
        wait_clock.add_sem_waits(
            drain_inst.ins, ScopedClock({None: tick_clock.global_clock})
        )
        si = drain_inst.ins.sync_info
        waits = list(si.on_wait) if si is not None else []
        if len(waits) > 1:
            si.on_wait = [waits[0]]
            by_name = {h.name: h for h in self.sems.allocated().values()}
            for w in waits[1:]:
                self.nc.sync.wait_ge(by_name[w.ant_name], w.wait_value)

        self.nc.all_engine_barrier()
        assert self.sems is not None
        popped = self.nc._tile_sem_poison_stack.pop()
        assert popped is self._sem_poison
        self.nc.clear_and_free_semaphores(list(self.sems.allocated().values()))
        # No trailing all_engine_barrier: the walrus epilogue that follows
        # re-zeroes every semaphore per-engine and ends in its own global
        # barrier, so the isolation barrier here only adds ~0.3us.

    _drain_and_barrier._single_wait_patch = True
    tile.TileContext._drain_and_barrier = _drain_and_barrier


def _build_nc():
    import concourse.bass as bass
    import concourse.tile as tile
    from concourse import mybir
    from concourse.bass import MemorySpace

    _patch_tile_drain()

    f32 = mybir.dt.float32
    X = mybir.AxisListType.X

    XY = mybir.AxisListType.XY
    NC_ = PPART + ROWS + NWHOLE  # consts free dim: ident | w1 | maskA

    nc = bass.Bass("TRN2")
    ptm = nc.dram_tensor("ptm", [PPART, HPP, HALF], f32, kind="ExternalInput")
    consts = nc.dram_tensor("consts", [PPART, NC_], f32, kind="ExternalInput")
    m1 = nc.dram_tensor("m1", [1, 2 * ROWS + ROWS * NMEM], f32, kind="ExternalInput")
    out = nc.dram_tensor("out", [1, ROWS], f32, kind="ExternalOutput")

    with tile.TileContext(nc) as tc:
        with (
            tc.tile_pool(name="data", bufs=1) as dpool,
            tc.tile_pool(name="small", bufs=1) as spool,
            tc.tile_pool(name="psum", bufs=1, space=MemorySpace.PSUM) as ppool,
        ):
            # Constants / small inputs: TWO dmas total. More small DMAs means
            # hundreds of tiny per-partition descriptors that round-robin
            # against the big stream's packets and take >15us to drain.
            const_t = spool.tile([PPART, NC_], f32)
            nc.scalar.dma_start(out=const_t[:], in_=consts[:])
            m1_t = spool.tile([1, 2 * ROWS + ROWS * NMEM], f32)
            nc.scalar.dma_start(out=m1_t[:], in_=m1[:])
            ident_v = const_t[:, 0:PPART]
            w1_v = const_t[:, PPART : PPART + ROWS]
            maskA_v = const_t[:, PPART + ROWS : NC_]
            maskS2_v = m1_t[0:1, 0 : 2 * ROWS]
            maskT1_v = m1_t[0:1, 2 * ROWS :]

            # PE warmup: touch the const tile once so later PE ops carry a
            # single data wait (walrus one-wait-per-instruction limit).
            warm = ppool.tile([1, ROWS], f32)
            nc.tensor.matmul(warm[:], const_t[:, 0:1], const_t[:, 0:ROWS],
                             start=True, stop=True)

            # Stray (half-segment) column load first; its processing
            # overlaps the stream.
            dS = dpool.tile([PPART, 1, HALF], f32, name="dataS", tag="dataS")
            nc.gpsimd.dma_start(out=dS[:], in_=ptm[:, HPP - 1 : HPP, :])
            statS = spool.tile([PPART, 1], f32)
            nc.vector.reduce_max(out=statS[:], in_=dS[:], axis=X)

            # Row counts in transposed (1, 32) layout, off critical path.
            m3 = maskT1_v.rearrange("one (r m) -> one r m", m=NMEM)
            cntT = spool.tile([1, ROWS], f32)
            nc.vector.reduce_sum(out=cntT[:], in_=m3, axis=X)
            rcntT = spool.tile([1, ROWS], f32)
            nc.vector.reciprocal(out=rcntT[:], in_=cntT[:])
            # DVE warm-touch of the const tile (maskA user below), plus a
            # burn op during the pre-stream idle window: some runs start with
            # the DVE ~20% slow (6.6us vs 5.5us per w=2 reduce); if that is a
            # power-state ramp, a few thousand early cycles absorb it here
            # instead of in the reduce pipeline.
            touch = spool.tile([1, 1], f32)
            nc.vector.tensor_copy(out=touch[:], in_=const_t[0:1, 0:1])
            burn = spool.tile([PPART, 1], f32)
            nc.vector.reduce_max(out=burn[:], in_=const_t[:], axis=X)
            nc.vector.reduce_max(out=burn[:], in_=const_t[:], axis=X)

            # PE transpose of the stray half-maxes to one partition.
            strayP = ppool.tile([1, PPART], f32)
            nc.tensor.transpose(strayP[:], statS[:], ident_v)

            # Main loads: 7 col-pair loads (one whole segment per partition
            # per load). X-axis w=2 reduces (2765 ns/col; the XY variant
            # measured 3331 ns/col) into half-maxes; the halves are joined by
            # one stride-2 pairmax at the end.
            # The final segment is loaded as two w=1 halves: its half-maxes
            # arrive ~1.5us earlier than a single w=2 load's completion, and
            # the end of the pipeline is delivery-gated.
            stats13 = spool.tile([PPART, 2 * NWHOLE], f32)
            straysum = spool.tile([1, ROWS], f32)
            plan = [(2 * k, 2) for k in range(NWHOLE - 1)]
            plan += [(2 * NWHOLE - 2, 1), (2 * NWHOLE - 1, 1)]
            for t, (col, w) in enumerate(plan):
                d = dpool.tile(
                    [PPART, w, HALF],
                    f32,
                    name=f"data{w}",
                    tag=f"data{w}",
                    bufs=sum(1 for _, ww in plan if ww == w),
                )
                nc.gpsimd.dma_start(out=d[:], in_=ptm[:, col : col + w, :])
                nc.vector.reduce_max(
                    out=stats13[:, col : col + w], in_=d[:], axis=X
                )
                if t == 1:
                    # Stray path on DVE, inserted mid-stream: copy the PSUM
                    # transpose to SBUF (TT may read only one PSUM input),
                    # then pairwise max, mask, pair-sum -> straysum (1,32).
                    strayC = spool.tile([1, PPART], f32)
                    nc.vector.tensor_copy(out=strayC[:], in_=strayP[:])
                    strayM = spool.tile([1, 2 * ROWS], f32)
                    nc.vector.tensor_max(
                        out=strayM[:],
                        in0=strayC[0:1, 0 : PPART : 2],
                        in1=strayC[0:1, 1 : PPART : 2],
                    )
                    strayMM = spool.tile([1, 2 * ROWS], f32)
                    nc.vector.tensor_mul(
                        out=strayMM[:], in0=strayM[:], in1=maskS2_v
                    )
                    sv = strayMM[:].rearrange("one (r two) -> one r two", two=2)
                    nc.vector.reduce_sum(out=straysum[:], in_=sv, axis=X)

            # Tail: pairmax -> mask -> row partials -> PE matmul -> out.
            segmax = spool.tile([PPART, NWHOLE], f32)
            nc.vector.tensor_max(
                out=segmax[:],
                in0=stats13[:, 0 : 2 * NWHOLE : 2],
                in1=stats13[:, 1 : 2 * NWHOLE : 2],
            )
            masked = spool.tile([PPART, NWHOLE], f32)
            nc.vector.tensor_mul(out=masked[:], in0=segmax[:], in1=maskA_v)
            partial = spool.tile([PPART, 1], f32)
            nc.vector.reduce_sum(out=partial[:], in_=masked[:], axis=X)

            acc = ppool.tile([1, ROWS], f32)
            nc.tensor.matmul(acc[:], partial[:], w1_v, start=True, stop=True)

            tmp = spool.tile([1, ROWS], f32)
            nc.vector.tensor_add(out=tmp[:], in0=acc[:], in1=straysum[:])
            res = spool.tile([1, ROWS], f32)
            nc.vector.tensor_mul(out=res[:], in0=tmp[:], in1=rcntT[:])
            nc.scalar.dma_start(out=out[:], in_=res[:])

    return nc


def _get_nc():
    if "nc" not in _NC_CACHE:
        _NC_CACHE["nc"] = _build_nc()
    return _NC_CACHE["nc"]


def _host_layout():
    """Pair-aligned half-segment permutation and mask/weight constants.

    idx[p, j] = half-segment index (seg*2 + half, within one core's 1920)
    placed at (partition p, col j). Row r owns partitions 4r..4r+3; each
    holds 7 whole segments (cols 0..13, halves adjacent) plus one stray
    half at col 14 (segs 28/29 of the row, halves on partition pairs).
    """
    idx = np.empty((PPART, HPP), dtype=np.int64)
    w1row = np.zeros((PPART, ROWS), dtype=np.float32)
    for r in range(ROWS):
        for j in range(4):
            p = 4 * r + j
            w1row[p, r] = 1.0
            for k in range(NWHOLE):
                seg = r * NMEM + 7 * j + k
                idx[p, 2 * k] = 2 * seg
                idx[p, 2 * k + 1] = 2 * seg + 1
        idx[4 * r + 0, 14] = 2 * (r * NMEM + 28)
        idx[4 * r + 1, 14] = 2 * (r * NMEM + 28) + 1
        idx[4 * r + 2, 14] = 2 * (r * NMEM + 29)
        idx[4 * r + 3, 14] = 2 * (r * NMEM + 29) + 1
    ident = np.eye(PPART, dtype=np.float32)
    return idx.reshape(-1), w1row, ident


_IDX, _W1ROW, _IDENT = _host_layout()


def make_in_maps(ptm, mem_mask):
    ptm = np.ascontiguousarray(np.asarray(ptm, dtype=np.float32))
    mask = np.asarray(mem_mask).reshape(N, NMEM).astype(np.float32)
    halves = ptm.reshape(N * NMEM * 2, HALF)

    in_maps = []
    for i in range(NCORES):
        core_halves = halves[i * SEGS * 2 : (i + 1) * SEGS * 2]
        shard = core_halves[_IDX].reshape(PPART, HPP, HALF)
        m = mask[i * ROWS : (i + 1) * ROWS]  # (32, 30)
        maskA = np.empty((PPART, NWHOLE), dtype=np.float32)
        for j in range(4):
            maskA[j::4] = m[:, 7 * j : 7 * j + 7]
        # consts = ident | w1 | maskA side by side: one DMA, few descriptors.
        consts = np.concatenate([_IDENT, _W1ROW, maskA], axis=1)
        consts = np.ascontiguousarray(consts, dtype=np.float32)
        # m1 = maskS2 | maskT1 on one partition: one DMA.
        m1 = np.concatenate(
            [m[:, 28:30].reshape(-1), m.reshape(-1)]
        ).reshape(1, -1).astype(np.float32)
        in_maps.append({"ptm": shard, "consts": consts, "m1": np.ascontiguousarray(m1)})
    return in_maps


def _ensure_ntff_hook():
    """Register the axon NTFF profiling hook (the container's antenv lacks
    axon_hooks; synthesize it from trn_agent_boot), and stub the artifact
    upload which has no bucket access here."""
    import types

    try:
        from antenv.axon_hooks import get_axon_ntff_profile_hook  # noqa: F401
    except ImportError:
        import antenv
        from trn_agent_boot.trn_boot import _ntff_profile_via_ctypes

        mod = types.ModuleType("antenv.axon_hooks")
        mod._hook = _ntff_profile_via_ctypes("/opt/axon/libaxon_pjrt.so")
        mod.set_axon_ntff_profile_hook = lambda h: setattr(mod, "_hook", h)
        mod.get_axon_ntff_profile_hook = lambda: mod._hook
        sys.modules["antenv.axon_hooks"] = mod
        antenv.axon_hooks = mod

    from concourse import bass_utils

    if not getattr(bass_utils.upload_artifacts, "_stubbed", False):
        def _no_upload(tmpdir):
            return str(tmpdir)

        _no_upload._stubbed = True
        bass_utils.upload_artifacts = _no_upload


def run(ptm, mem_mask, trace=False):
    from concourse.bass_utils import run_bass_kernel_spmd

    if trace:
        _ensure_ntff_hook()

    in_maps = make_in_maps(ptm, mem_mask)

    nc = _get_nc()
    kr = run_bass_kernel_spmd(nc, in_maps, list(range(NCORES)), trace=trace)
    out = np.concatenate(
        [np.asarray(kr.results[i]["out"]).reshape(ROWS) for i in range(NCORES)]
    )
    return out.astype(np.float32), kr


def kernel(ptm, mem_mask):
    out, _ = run(ptm, mem_mask, trace=False)
    return out

